# revision 9
# baseline (speedup 1.0000x reference)
"""MemoryController fused kernel for 8 TRN2 NeuronCores.

Sharding: 16384 tokens (B*S) split 2048/core; the cache slice for layer 3 is
per-batch (core c handles batch c//2) and is replicated to the cores that
need it. Everything is token-parallel, so no collectives are required.

Math (per token t):
  hn = (x - mean)*rstd                      (LayerNorm without affine)
  gate1T[j,t] = silu(sum_d hn[d,t]*G1p[d,j] + bias1[j])   G1p = ln_w*g1_w
  logit[t] = sum_j g2[j]*gate1T[j,t]        gate = (logit > -g2_b)
  q = (x @ rq_w + rq_b)/16                  scores = qT.T @ contentT
  attn = softmax(clip(scores, +-20))
  context = attn @ (local @ fc_w + fc_b)    (associativity: rows of attn sum to 1)
  x_enh = x + gate*context

Gate path runs in fp32 (hard threshold: min |logit| over the dataset is 3.6e-5,
fp32r's ~1.6e-4 matmul error would flip gates). The context matmul runs in
fp32r (full PE rate); scores/query stay fp32 so attn is fp32-exact.
"""

import numpy as np

import concourse.bacc as bacc
import concourse.mybir as mybir
from concourse.tile import TileContext
from concourse.masks import make_identity
from concourse.bass_utils import run_bass_kernel_spmd

F32 = mybir.dt.float32
F32R = mybir.dt.float32r
AF = mybir.ActivationFunctionType
ALU = mybir.AluOpType

B, S, D = 4, 4096, 2048
DC, NSLOT, DSLOT = 256, 128, 273
LAYER_START = 3 * 128
N_CORES = 8
T_CORE = (B * S) // N_CORES  # 2048 tokens per core
CHUNK = 256                  # tokens per pipeline chunk
N_CHUNKS = T_CORE // CHUNK
NSUB = CHUNK // 128          # 128-token subtiles per chunk
KD = D // 128                # 16 contraction chunks over d_model
MJ = 1024 // 128             # 8 chunks over gate hidden dim

LN_EPS = 1e-5
TRACE = False  # set by test harness for profiling runs
_BUILT = None


def _build(g2_bias: float):
    nc = bacc.Bacc()

    xin = nc.declare_dram_parameter("xin", [T_CORE, D], F32, isOutput=False)
    g1p = nc.declare_dram_parameter("g1p", [D, 1024], F32, isOutput=False)
    bias1 = nc.declare_dram_parameter("bias1", [MJ, 128, 1], F32, isOutput=False)
    g2w = nc.declare_dram_parameter("g2w", [1024, 1], F32, isOutput=False)
    rqw = nc.declare_dram_parameter("rqw", [D, DC], F32, isOutput=False)
    rqb16 = nc.declare_dram_parameter("rqb16", [2, 128, 1], F32, isOutput=False)
    local = nc.declare_dram_parameter("local", [NSLOT, DSLOT], F32, isOutput=False)
    fcaug = nc.declare_dram_parameter("fcaug", [DSLOT + 1, D], F32, isOutput=False)

    xenh_o = nc.declare_dram_parameter("xenh", [T_CORE, D], F32, isOutput=True)
    ctx_o = nc.declare_dram_parameter("ctx", [T_CORE, D], F32, isOutput=True)
    attn_o = nc.declare_dram_parameter("attn", [T_CORE, NSLOT], F32, isOutput=True)

    with TileContext(nc) as tc:
        with tc.tile_pool(name="wpool", bufs=1) as wpool:
            ident = wpool.tile([128, 128], F32)
            make_identity(nc, ident[:])
            epst = wpool.tile([128, 1], F32)
            nc.vector.memset(epst[:], LN_EPS)
            one11 = wpool.tile([1, 1], F32)
            nc.vector.memset(one11[:], 1.0)

            # ---- resident weights ----
            g1t = []
            for k in range(KD):
                t = wpool.tile([128, 1024], F32, name=f"g1t{k}")
                nc.sync.dma_start(out=t[:], in_=g1p[k * 128 : (k + 1) * 128, :])
                g1t.append(t)
            g2t = []
            for m in range(MJ):
                t = wpool.tile([128, 1], F32, name=f"g2t{m}")
                nc.sync.dma_start(out=t[:], in_=g2w[m * 128 : (m + 1) * 128, :])
                g2t.append(t)
            b1t = []
            for m in range(MJ):
                t = wpool.tile([128, 1], F32, name=f"b1t{m}")
                nc.sync.dma_start(out=t[:], in_=bias1[m])
                b1t.append(t)
            rqwt = []
            for k in range(KD):
                t = wpool.tile([128, DC], F32, name=f"rqwt{k}")
                nc.sync.dma_start(out=t[:], in_=rqw[k * 128 : (k + 1) * 128, :])
                rqwt.append(t)
            rqbt = []
            for m in range(2):
                t = wpool.tile([128, 1], F32, name=f"rqbt{m}")
                nc.sync.dma_start(out=t[:], in_=rqb16[m])
                rqbt.append(t)
            # contentT (fp32) for scores; fc_local (fp32r) for context
            locT_f = [
                wpool.tile([128, 128], F32, name=f"locTf{i}") for i in range(2)
            ]
            fcl_r = wpool.tile([128, D], F32R, name="fcl_r")

            # ---- prep: localT + fc_local (transient pools) ----
            with (
                tc.tile_pool(name="prpool", bufs=1) as prpool,
                tc.tile_pool(name="prpsum", space="PSUM", bufs=2) as prpsum,
                tc.tile_pool(name="fcpsum", space="PSUM", bufs=1) as fcpsum,
            ):
                loc_sb = prpool.tile([128, DSLOT], F32)
                nc.sync.dma_start(out=loc_sb[:], in_=local[:, :])
                locT_r = [
                    prpool.tile([128, 128], F32R, name=f"locTr{i}") for i in range(2)
                ]
                locT_r.append(prpool.tile([18, 128], F32R, name="locTr2"))
                for i in range(3):
                    w = 128 if i < 2 else DSLOT - 256
                    pst = prpsum.tile([128, 128], F32, name="pst", tag="pst")
                    nc.tensor.transpose(
                        pst[:w, :], loc_sb[:, i * 128 : i * 128 + w], ident[:]
                    )
                    if i < 2:
                        nc.scalar.copy(locT_f[i][:], pst[:, :])
                        nc.vector.tensor_copy(locT_r[i][:], pst[:, :])
                    else:
                        ones18 = prpool.tile([18, 128], F32, name="ones18")
                        nc.vector.memset(ones18[:], 1.0)
                        nc.vector.tensor_copy(locT_r[2][:, :], ones18[:])
                        nc.vector.tensor_copy(locT_r[2][:w, :], pst[:w, :])

                # fc_local = [local, 1] @ [fc_w; fc_b] -> [128 slots, 2048]
                psf = [
                    fcpsum.tile([128, 512], F32, name=f"psf{n}") for n in range(4)
                ]
                for kc in range(3):
                    rows = 128 if kc < 2 else DSLOT + 1 - 256
                    fcw_sb = prpool.tile(
                        [128, D], F32, name="fcw_sb", tag="fcw_sb", bufs=2
                    )
                    nc.sync.dma_start(
                        out=fcw_sb[:rows, :],
                        in_=fcaug[kc * 128 : kc * 128 + rows, :],
                    )
                    fcw_r = prpool.tile(
                        [128, D], F32R, name="fcw_r", tag="fcw_r", bufs=2
                    )
                    nc.vector.tensor_copy(fcw_r[:rows, :], fcw_sb[:rows, :])
                    for n in range(4):
                        nc.tensor.matmul(
                            psf[n][:],
                            locT_r[kc][:rows, :],
                            fcw_r[:rows, n * 512 : (n + 1) * 512],
                            start=(kc == 0),
                            stop=(kc == 2),
                        )
                for n in range(4):
                    nc.vector.tensor_copy(fcl_r[:, n * 512 : (n + 1) * 512], psf[n][:])

            # ---- main token pipeline ----
            with (
                tc.tile_pool(name="xpool", bufs=3) as xpool,
                tc.tile_pool(name="hpool", bufs=2) as hpool,
                tc.tile_pool(name="tpool", bufs=1) as tpool,
                tc.tile_pool(name="spool", bufs=2) as spool,
                tc.tile_pool(name="opool", bufs=3) as opool,
                tc.tile_pool(name="pst_p", space="PSUM", bufs=2) as pst_p,
                tc.tile_pool(name="psg_p", space="PSUM", bufs=2) as psg_p,
                tc.tile_pool(name="psq_p", space="PSUM", bufs=2) as psq_p,
                tc.tile_pool(name="psc_p", space="PSUM", bufs=2) as psc_p,
            ):
                for c in range(N_CHUNKS):
                    t0 = c * CHUNK
                    xt = [
                        xpool.tile([128, D], F32, name="xt", tag="xt")
                        for _ in range(NSUB)
                    ]
                    for s in range(NSUB):
                        nc.sync.dma_start(
                            out=xt[s][:],
                            in_=xin[t0 + s * 128 : t0 + (s + 1) * 128, :],
                        )

                    # LayerNorm stats
                    rstd, nmr = [], []
                    for s in range(NSUB):
                        st6 = spool.tile([128, 4, 6], F32, name="st6", tag="st6")
                        for a in range(4):
                            nc.vector.bn_stats(
                                st6[:, a, :], xt[s][:, a * 512 : (a + 1) * 512]
                            )
                        mv = spool.tile([128, 2], F32, name="mv", tag="mv")
                        nc.vector.bn_aggr(mv[:], st6[:].rearrange("p a b -> p (a b)"))
                        std = spool.tile([128, 1], F32, name="std", tag="std")
                        nc.scalar.activation(std[:], mv[:, 1:2], AF.Sqrt, bias=epst[:])
                        rs = spool.tile([128, 1], F32, name="rs", tag="rs")
                        nc.vector.reciprocal(rs[:], std[:])
                        nm = spool.tile([128, 1], F32, name="nm", tag="nm")
                        nc.vector.scalar_tensor_tensor(
                            nm[:], mv[:, 0:1], -1.0, rs[:], op0=ALU.mult, op1=ALU.mult
                        )
                        rstd.append(rs)
                        nmr.append(nm)

                    # hn = (x - mu) * rstd  (token-major)
                    hn = [
                        hpool.tile([128, D], F32, name="hn", tag="hn")
                        for _ in range(NSUB)
                    ]
                    for s in range(NSUB):
                        nc.scalar.activation(
                            hn[s][:], xt[s][:], AF.Identity,
                            bias=nmr[s][:], scale=rstd[s][:],
                        )

                    # transposes: xT (for rq) and hnT (for g1), feature-major
                    xT = tpool.tile([128, KD * CHUNK], F32, name="xT", tag="xT")
                    hnT = tpool.tile([128, KD * CHUNK], F32, name="hnT", tag="hnT")
                    for src, dst, ceng in ((xt, xT, "v"), (hn, hnT, "a")):
                        for kk in range(KD // 2):
                            pst = pst_p.tile([128, 512], F32, name="pst", tag="pst")
                            for j in range(2):
                                k = kk * 2 + j
                                for s in range(NSUB):
                                    nc.tensor.transpose(
                                        pst[
                                            :,
                                            (j * NSUB + s) * 128 : (j * NSUB + s + 1)
                                            * 128,
                                        ],
                                        src[s][:, k * 128 : (k + 1) * 128],
                                        ident[:],
                                    )
                            if ceng == "v":
                                nc.vector.tensor_copy(
                                    dst[:, kk * 512 : (kk + 1) * 512], pst[:]
                                )
                            else:
                                nc.scalar.copy(
                                    dst[:, kk * 512 : (kk + 1) * 512], pst[:]
                                )

                    # gate MLP layer 1 (fp32): gate1T = silu(G1p.T @ hnT + bias1)
                    gate1T = [
                        spool.tile([128, CHUNK], F32, name="g1o", tag=f"g1o{m}")
                        for m in range(MJ)
                    ]
                    for m in range(MJ):
                        psg = psg_p.tile([128, CHUNK], F32, name="psg", tag="psg")
                        for k in range(KD):
                            nc.tensor.matmul(
                                psg[:],
                                g1t[k][:, m * 128 : (m + 1) * 128],
                                hnT[:, k * CHUNK : (k + 1) * CHUNK],
                                start=(k == 0),
                                stop=(k == KD - 1),
                            )
                        nc.scalar.activation(
                            gate1T[m][:], psg[:], AF.Silu, bias=b1t[m][:]
                        )

                    # gate MLP layer 2 (fp32) + threshold
                    psl = psq_p.tile([1, CHUNK], F32, name="psl", tag="qshare")
                    for m in range(MJ):
                        nc.tensor.matmul(
                            psl[:],
                            g2t[m][:],
                            gate1T[m][:],
                            start=(m == 0),
                            stop=(m == MJ - 1),
                        )
                    grow = spool.tile([1, CHUNK], F32, name="grow", tag="grow")
                    nc.vector.tensor_scalar(
                        grow[:], psl[:], -g2_bias, None, op0=ALU.is_gt
                    )
                    # move gate to per-token partitions: out[m,0] = grow[0,m] * 1
                    gate = []
                    for s in range(NSUB):
                        psgt = psq_p.tile([128, 1], F32, name="psgt", tag="qshare")
                        nc.tensor.matmul(
                            psgt[:],
                            grow[0:1, s * 128 : (s + 1) * 128],
                            one11[:],
                            start=True,
                            stop=True,
                        )
                        gp = spool.tile([128, 1], F32, name="gp", tag=f"gp{s}")
                        nc.vector.tensor_copy(gp[:], psgt[:])
                        gate.append(gp)

                    # read query (fp32): queryT = rq_w.T @ xT, + bias, /16
                    queryT = [
                        spool.tile([128, CHUNK], F32, name="qT", tag=f"qT{mc}")
                        for mc in range(2)
                    ]
                    for mc in range(2):
                        psq = psq_p.tile([128, CHUNK], F32, name="psq", tag="qshare")
                        for k in range(KD):
                            nc.tensor.matmul(
                                psq[:],
                                rqwt[k][:, mc * 128 : (mc + 1) * 128],
                                xT[:, k * CHUNK : (k + 1) * CHUNK],
                                start=(k == 0),
                                stop=(k == KD - 1),
                            )
                        nc.scalar.activation(
                            queryT[mc][:], psq[:], AF.Identity,
                            bias=rqbt[mc][:], scale=1.0 / 16.0,
                        )

                    # scores + softmax (fp32), then attnT in fp32r
                    attnT_r = spool.tile(
                        [128, CHUNK], F32R, name="attnT", tag="attnT"
                    )
                    for s in range(NSUB):
                        pss = pst_p.tile([128, 128], F32, name="pss", tag="pst")
                        for kc in range(2):
                            nc.tensor.matmul(
                                pss[:],
                                queryT[kc][:, s * 128 : (s + 1) * 128],
                                locT_f[kc][:],
                                start=(kc == 0),
                                stop=(kc == 1),
                            )
                        sc = spool.tile([128, 128], F32, name="sc", tag="sc")
                        nc.vector.tensor_scalar(
                            sc[:], pss[:], 20.0, -20.0, op0=ALU.min, op1=ALU.max
                        )
                        ex = spool.tile([128, 128], F32, name="ex", tag="ex")
                        sumex = spool.tile([128, 1], F32, name="sumex", tag="sumex")
                        nc.scalar.activation(ex[:], sc[:], AF.Exp, accum_out=sumex[:])
                        rsum = spool.tile([128, 1], F32, name="rsum", tag="rsum")
                        nc.vector.reciprocal(rsum[:], sumex[:])
                        at = spool.tile([128, 128], F32, name="at", tag="at")
                        nc.vector.tensor_scalar(
                            at[:], ex[:], rsum[:], None, op0=ALU.mult
                        )
                        nc.sync.dma_start(
                            out=attn_o[t0 + s * 128 : t0 + (s + 1) * 128, :],
                            in_=at[:],
                        )
                        psat = pst_p.tile([128, 128], F32, name="psat", tag="pst")
                        nc.tensor.transpose(psat[:], at[:], ident[:])
                        nc.vector.tensor_copy(
                            attnT_r[:, s * 128 : (s + 1) * 128], psat[:]
                        )

                    # context (fp32r) + epilogue
                    for s in range(NSUB):
                        for n in range(4):
                            psc = psc_p.tile([128, 512], F32, name="psc", tag="psc")
                            nc.tensor.matmul(
                                psc[:],
                                attnT_r[:, s * 128 : (s + 1) * 128],
                                fcl_r[:, n * 512 : (n + 1) * 512],
                                start=True,
                                stop=True,
                            )
                            ctx_sb = opool.tile(
                                [128, 512], F32, name="ctx_sb", tag="ctx_sb"
                            )
                            nc.scalar.copy(ctx_sb[:], psc[:])
                            nc.sync.dma_start(
                                out=ctx_o[
                                    t0 + s * 128 : t0 + (s + 1) * 128,
                                    n * 512 : (n + 1) * 512,
                                ],
                                in_=ctx_sb[:],
                            )
                            xe_sb = opool.tile(
                                [128, 512], F32, name="xe_sb", tag="xe_sb"
                            )
                            nc.vector.scalar_tensor_tensor(
                                xe_sb[:],
                                psc[:],
                                gate[s][:],
                                xt[s][:, n * 512 : (n + 1) * 512],
                                op0=ALU.mult,
                                op1=ALU.add,
                            )
                            nc.sync.dma_start(
                                out=xenh_o[
                                    t0 + s * 128 : t0 + (s + 1) * 128,
                                    n * 512 : (n + 1) * 512,
                                ],
                                in_=xe_sb[:],
                            )

    nc.finalize()
    return nc


def kernel(x, cache, ln_w, ln_b, g1_w, g1_b, g2_w, g2_b, rq_w, rq_b, fc_w, fc_b):
    global _BUILT
    x = np.ascontiguousarray(np.asarray(x, dtype=np.float32))
    cache = np.asarray(cache, dtype=np.float32)

    g1p = (np.asarray(ln_w)[:, None] * np.asarray(g1_w)).astype(np.float32)
    bias1 = (np.asarray(ln_b) @ np.asarray(g1_w) + np.asarray(g1_b)).astype(np.float32)
    fcaug = np.concatenate(
        [np.asarray(fc_w), np.asarray(fc_b)[None, :]], axis=0
    ).astype(np.float32)
    rqb16 = (np.asarray(rq_b) / 16.0).astype(np.float32)
    g2b = float(np.asarray(g2_b).reshape(-1)[0])

    if _BUILT is None:
        _BUILT = _build(g2b)
    nc = _BUILT

    xflat = x.reshape(B * S, D)
    common = {
        "g1p": g1p,
        "bias1": bias1.reshape(MJ, 128, 1),
        "g2w": np.ascontiguousarray(np.asarray(g2_w, dtype=np.float32).reshape(1024, 1)),
        "rqw": np.ascontiguousarray(np.asarray(rq_w, dtype=np.float32)),
        "rqb16": rqb16.reshape(2, 128, 1),
        "fcaug": fcaug,
    }
    in_maps = []
    for c in range(N_CORES):
        b = c // (N_CORES // B)
        in_maps.append(
            dict(
                common,
                xin=np.ascontiguousarray(xflat[c * T_CORE : (c + 1) * T_CORE]),
                local=np.ascontiguousarray(
                    cache[b, LAYER_START : LAYER_START + NSLOT, :]
                ),
            )
        )

    res = run_bass_kernel_spmd(nc, in_maps, list(range(N_CORES)), trace=TRACE)
    if TRACE and res.exec_time_ns is not None:
        print(f"HW exec time: {res.exec_time_ns} ns")
        print(f"HW exec time mean: {res.mean_exec_time_ns} ns")

    x_enh = np.concatenate([r["xenh"] for r in res.results], axis=0).reshape(B, S, D)
    context = np.concatenate([r["ctx"] for r in res.results], axis=0).reshape(B, S, D)
    attn = np.concatenate([r["attn"] for r in res.results], axis=0).reshape(
        B, S, NSLOT
    )
    return (x_enh, context, attn)
